# revision 10
# baseline (speedup 1.0000x reference)
"""Causal self-attention (B=4, T=2048, D=1024, H=16) on 8 Trainium2 NeuronCores.

Sharding: batch x head-half. Core c handles batch b = c//2 and heads
hh..hh+7 where hh = 8*(c%2)  (tensor-parallel split of w_qkv output dim and
w_o input dim). Each core produces a partial o_proj output [2048, 1024];
the host sums the two partials per batch (the 2-way all-reduce).

Per-core kernel (all matmuls bf16, fp32 PSUM accumulate). Head pairs are
fused into [128, 1024] two-bank PSUM tiles throughout so that ACT/DVE/DMA
process both heads with one instruction:
  phase 1: qkv projection. Q^T,K^T produced head-pair-stacked [128, t] for
           row-tiled score matmuls; V produced in natural [t, dk] layout with
           an appended ones column (row-sum trick).
  phase 2: causal attention per (q-tile of 512, head-pair): S^T for both
           heads of the pair via two row-tiled (K=64) matmuls into one
           psum pair-tile, one exp on ACT (scale 1/8 folded in), causal
           triangle masking on diagonal chunks via a static mask multiply,
           PV+rowsum matmuls ([128,65] stationary) accumulating into a pv
           pair-tile, then normalize both heads by 1/rowsum at once.
  phase 3: o_proj partial (interleaved per q-tile).
"""
import numpy as np
import ml_dtypes

B, T, D, H = 4, 2048, 1024, 16
DK = D // H          # 64
HPC = 8              # heads per core
NCORES = 8
NQT = T // 512       # 4
NKC = T // 128       # 16

_cache = {}


def _emit(nc, tc, pools, dram, opts=()):
    import concourse.mybir as mybir

    bf16 = mybir.dt.bfloat16
    f32 = mybir.dt.float32
    Exp = mybir.ActivationFunctionType.Exp
    cst, big, work, stp, psa, psb = pools
    xt_d, wq_d, wo_d, out_d, masks = dram

    # per-d-chunk input tiles so compute can start before the full load
    xts = [big.tile([128, T], bf16, tag=f"xt{dc}", name=f"xts{dc}")
           for dc in range(8)]
    wqs = [big.tile([128, 1536], bf16, tag=f"wq{dc}", name=f"wqs{dc}")
           for dc in range(8)]
    wos = [big.tile([128, D], bf16, tag=f"wo{pr}", name=f"wos{pr}")
           for pr in range(4)]
    qk = big.tile([128, 8, T], bf16, tag="qk")
    vt = big.tile([128, NKC, HPC, DK + 1], bf16, tag="vt")
    ob = big.tile([128, 4, T], bf16, tag="ob")

    for dc in range(8):
        nc.sync.dma_start(xts[dc][:], xt_d[dc])
        nc.scalar.dma_start(wqs[dc][:], wq_d[dc])
    for pr in range(4):
        nc.scalar.dma_start(wos[pr][:], wo_d[pr])

    nc.gpsimd.memset(vt[:, :, :, DK], 1.0)

    # ---- phase 1a: Q^T / K^T  (pairs of t-chunks share a 2-bank psum;
    # alternate between the two psum pools for 4-deep pipelining) ----
    for ec in range(8):
        for tp2 in range(2):
            g = 2 * ec + tp2
            ps = (psa.tile([128, 1024], f32, tag="s", name=f"qps{g}")
                  if g % 2 == 0 else
                  psb.tile([128, 1024], f32, tag="pv", name=f"qps{g}"))
            for half in range(2):
                tcx = 2 * tp2 + half
                for dc in range(8):
                    nc.tensor.matmul(
                        ps[:, half * 512:(half + 1) * 512],
                        wqs[dc][:, ec * 128:(ec + 1) * 128],
                        xts[dc][:, tcx * 512:(tcx + 1) * 512],
                        start=(dc == 0), stop=(dc == 7),
                    )
            nc.vector.tensor_copy(qk[:, ec, tp2 * 1024:(tp2 + 1) * 1024], ps[:])

    # ---- phase 1b: V (natural layout, scattered by head) ----
    for tq in range(8):
        ps = (psa.tile([128, 1024], f32, tag="s", name=f"vps{tq}")
              if tq % 2 == 0 else
              psb.tile([128, 1024], f32, tag="pv", name=f"vps{tq}"))
        for half in range(2):
            tt = 2 * tq + half
            for dc in range(8):
                nc.tensor.matmul(
                    ps[:, half * 512:(half + 1) * 512],
                    xts[dc][:, tt * 128:(tt + 1) * 128],
                    wqs[dc][:, 1024:1536],
                    start=(dc == 0), stop=(dc == 7),
                )
        nc.vector.tensor_copy(
            vt[:, 2 * tq:2 * tq + 2, :, 0:DK],
            ps[:].rearrange("p (t h d) -> p t h d", t=2, d=DK))

    # ---- phase 2: causal attention (qt outer so o_proj can overlap) ----
    for qt in range(NQT):
        for pr in range(4):          # head pair (2*pr, 2*pr+1)
            pvp = psb.tile([65, 1024], f32, tag="pv")
            nkc = 4 * qt + 4
            for kc in range(nkc):
                i = kc - 4 * qt
                lo = max(i, 0) * 128   # first unmasked column of this chunk
                sp = psa.tile([128, 1024], f32, tag="s")
                nc.tensor.matmul(
                    sp[:, lo:512], qk[0:64, 4 + pr, kc * 128:(kc + 1) * 128],
                    qk[0:64, pr, qt * 512 + lo:(qt + 1) * 512],
                    start=True, stop=True, tile_position=(0, 0))
                nc.tensor.matmul(
                    sp[:, 512 + lo:1024],
                    qk[64:128, 4 + pr, kc * 128:(kc + 1) * 128],
                    qk[64:128, pr, qt * 512 + lo:(qt + 1) * 512],
                    start=True, stop=True, tile_position=(64, 0))
                st = stp.tile([128, 1024], bf16, tag="st")
                sp3 = sp[:].rearrange("p (h q) -> p h q", h=2)[:, :, lo:]
                st3 = st[:].rearrange("p (h q) -> p h q", h=2)[:, :, lo:]
                nc.scalar.activation(st3, sp3, Exp, scale=0.125)
                if i >= 0:      # diagonal chunk: mask the triangular blocks
                    nc.vector.tensor_mul(
                        st[:, lo:lo + 128], st[:, lo:lo + 128], masks[:])
                    nc.vector.tensor_mul(
                        st[:, 512 + lo:512 + lo + 128],
                        st[:, 512 + lo:512 + lo + 128], masks[:])
                nc.tensor.matmul(
                    pvp[:, lo:512], vt[:, kc, 2 * pr, :], st[:, lo:512],
                    start=(kc == 0), stop=(kc == nkc - 1))
                nc.tensor.matmul(
                    pvp[:, 512 + lo:1024], vt[:, kc, 2 * pr + 1, :],
                    st[:, 512 + lo:1024],
                    start=(kc == 0), stop=(kc == nkc - 1))
            # normalize both heads: ob[:, pr, qt] = pv[0:64] * (1/rowsum)
            # (custom-DVE ops ignore the input AP's partition base, so stage
            # the sums row at partition 0 first)
            sd = work.tile([1, 1024], f32, tag="sd")
            nc.vector.tensor_copy(sd[:], pvp[64:65, :])
            rc = work.tile([1, 1024], f32, tag="rc")
            nc.vector.reciprocal_approx_fast(rc[:], sd[:])
            rb = work.tile([64, 1024], f32, tag="rb")
            nc.gpsimd.partition_broadcast(rb[:], rc[:])
            for hh in range(2):
                nc.vector.tensor_mul(
                    ob[64 * hh:64 * hh + 64, pr, qt * 512:(qt + 1) * 512],
                    pvp[0:64, hh * 512:(hh + 1) * 512],
                    rb[:, hh * 512:(hh + 1) * 512])

        # ---- phase 3 (interleaved): o_proj for the finished q-range ----
        for tt in range(4 * qt, 4 * qt + 4):
            po = psb.tile([128, 1024], f32, tag="pv")
            for eh in range(2):
                for pr in range(4):
                    nc.tensor.matmul(
                        po[:, eh * 512:(eh + 1) * 512],
                        ob[:, pr, tt * 128:(tt + 1) * 128],
                        wos[pr][:, eh * 512:(eh + 1) * 512],
                        start=(pr == 0), stop=(pr == 3))
            ot = work.tile([128, 1024], f32, tag="ot")
            nc.vector.tensor_copy(ot[:], po[:])
            nc.gpsimd.dma_start(out_d[tt * 128:(tt + 1) * 128, :], ot[:])


def _build(reps=1, opts=()):
    import concourse.mybir as mybir
    import concourse.tile as tile
    from concourse import bacc

    bf16 = mybir.dt.bfloat16
    f32 = mybir.dt.float32

    nc = bacc.Bacc("TRN2", target_bir_lowering=False, debug=False,
                   num_devices=NCORES)
    xt_d = nc.dram_tensor("xt", [8, 128, T], bf16, kind="ExternalInput")
    wq_d = nc.dram_tensor("wq", [8, 128, 1536], bf16, kind="ExternalInput")
    wo_d = nc.dram_tensor("wo", [4, 128, D], bf16, kind="ExternalInput")
    out_d = nc.dram_tensor("out", [T, D], f32, kind="ExternalOutput")

    with tile.TileContext(nc) as tc:
        with (
            tc.tile_pool(name="cst", bufs=1) as cst,
            tc.tile_pool(name="big", bufs=1) as big,
            tc.tile_pool(name="work", bufs=4) as work,
            tc.tile_pool(name="stp", bufs=4) as stp,
            tc.tile_pool(name="psa", bufs=2, space="PSUM") as psa,
            tc.tile_pool(name="psb", bufs=2, space="PSUM") as psb,
        ):
            # static causal mask for the 128x128 diagonal blocks:
            # masks[p, q] = 1 if q >= p else 0
            masks = cst.tile([128, 128], bf16)
            nc.gpsimd.memset(masks[:], 1.0)
            nc.gpsimd.affine_select(
                out=masks[:], in_=masks[:],
                compare_op=mybir.AluOpType.is_ge, fill=0.0,
                base=0, channel_multiplier=-1, pattern=[[1, 128]],
            )
            pools = (cst, big, work, stp, psa, psb)
            dram = (xt_d, wq_d, wo_d, out_d, masks)
            if reps == 1:
                _emit(nc, tc, pools, dram, opts)
            else:
                with tc.For_i(0, reps, 1):
                    _emit(nc, tc, pools, dram, opts)

    nc.compile()
    return nc


def prep_inputs(x, w_qkv, w_o):
    """Host-side shard + layout prep. Returns in_maps for cores 0..7."""
    bf = ml_dtypes.bfloat16
    in_maps = []
    for c in range(NCORES):
        b, hh = c // 2, HPC * (c % 2)
        qrows = w_qkv[hh * DK:(hh + HPC) * DK]                    # [512, 1024]
        krows = w_qkv[D + hh * DK:D + (hh + HPC) * DK]
        vrows = w_qkv[2 * D + hh * DK:2 * D + (hh + HPC) * DK]
        wqt = np.concatenate([qrows, krows, vrows], 0).T          # [1024, 1536]
        in_maps.append({
            "xt": np.ascontiguousarray(x[b].T).astype(bf).reshape(8, 128, T),
            "wq": wqt.astype(bf).reshape(8, 128, 1536),
            "wo": np.ascontiguousarray(w_o[:, hh * DK:(hh + HPC) * DK].T)
                    .astype(bf).reshape(4, 128, D),
        })
    return in_maps


def get_nc(reps=1, opts=()):
    key = ("nc", reps, tuple(opts))
    if key not in _cache:
        _cache[key] = _build(reps, tuple(opts))
    return _cache[key]


def kernel(x, w_qkv, w_o):
    from concourse.bass_utils import run_bass_kernel_spmd

    nc = get_nc()
    in_maps = prep_inputs(np.asarray(x, dtype=np.float32),
                          np.asarray(w_qkv, dtype=np.float32),
                          np.asarray(w_o, dtype=np.float32))
    res = run_bass_kernel_spmd(nc, in_maps, core_ids=list(range(NCORES)))
    out = np.empty((B, T, D), np.float32)
    for b in range(B):
        out[b] = res.results[2 * b]["out"] + res.results[2 * b + 1]["out"]
    return out
